# revision 10
# baseline (speedup 1.0000x reference)
"""EMA scan kernel for Trainium2 (Bass/Tile), 8-core SPMD.

Problem: h_t = (1-a)*y_t + a*h_{t-1}, h_{-1}=0, a=0.9, over y [B=4, S=4096, D=2048] f32.

Sharding: B(4) x D-half(2) -> 8 cores, each core handles a [S=4096, Dc=1024] slab.

Per-core algorithm (exact, matmul-based):
  Split S into 32 blocks of TB=128 rows. For block b:
      h_b = L @ y_b + M1 @ z_{b-1}
  where L[t,j]  = (1-a)*a^(t-j) for t>=j else 0          (in-block causal scan)
        M1[t,j] = (1-a)*a^(t+128-j)                      (previous-block window)
  and z_b = y_b + a^128 * z_{b-1} is a block-level EMA of the raw inputs.
  This is exact because the lag-(128m) window matrices satisfy M_m = a^(128(m-1)) * M1,
  so summing M_m @ y_{b-m} over all m telescopes into M1 @ z_{b-1}.

Both matmuls run on the PE (fp32, K=128, N=512 chunks, PSUM accumulate),
z-chain is one scalar_tensor_tensor per block on DVE, PSUM->SBUF copies are
split between DVE and ACT, DMA is batched 4 blocks (2 MiB) per transfer.
"""

import numpy as np

import concourse.bass as bass
import concourse.tile as tile
from concourse import bacc, mybir
from concourse import bass_utils

ALPHA = 0.9
B, S, D = 4, 4096, 2048
NCORES = 8
DC = D // 2          # per-core D chunk (1024)
TB = 128             # S-block size (partition dim)
NB = S // TB         # 32 blocks
GK = 4               # blocks per DMA group
NG = NB // GK        # 8 groups
NC_CHUNK = 512       # matmul moving-operand chunk (one PSUM bank, fp32)
F32 = mybir.dt.float32
F32R = mybir.dt.float32r  # tf32-class PE fast path (1 cyc/row vs 4 for fp32)


def _consts():
    a = ALPHA
    t = np.arange(TB)
    diff = t[:, None] - t[None, :]
    L = np.where(diff >= 0, (1.0 - a) * a ** np.maximum(diff, 0), 0.0)
    M1 = (1.0 - a) * a ** (t[:, None] + TB - t[None, :])
    LT = np.ascontiguousarray(L.T).astype(np.float32)
    M1T = np.ascontiguousarray(M1.T).astype(np.float32)
    c = float(a**TB)
    return LT, M1T, c


_CACHE = {}


def _build(ybufs=4, obufs=5, zbufs=3, psbufs=4, gk=GK, dve_frac=0, warmup=9, zsplit=4, out_gk=2):
    key = (ybufs, obufs, zbufs, psbufs, gk, dve_frac, warmup, zsplit, out_gk)
    if key in _CACHE:
        return _CACHE[key]
    _, _, c = _consts()
    GKL = gk
    NGL = NB // gk

    nc = bacc.Bacc(
        "TRN2",
        target_bir_lowering=False,
        debug=False,
        enable_asserts=False,
        num_devices=NCORES,
    )
    y_dram = nc.dram_tensor("y", [S, DC], F32, kind="ExternalInput")
    lt_dram = nc.dram_tensor("lt", [TB, TB], F32, kind="ExternalInput")
    m1t_dram = nc.dram_tensor("m1t", [TB, TB], F32, kind="ExternalInput")
    out_dram = nc.dram_tensor("out", [S, DC], F32, kind="ExternalOutput")

    with tile.TileContext(nc) as tc:
        with (
            tc.tile_pool(name="consts", bufs=1) as cpool,
            tc.tile_pool(name="ypool", bufs=ybufs) as ypool,
            tc.tile_pool(name="zpool", bufs=zbufs) as zpool,
            tc.tile_pool(name="opool", bufs=obufs) as opool,
            tc.tile_pool(name="psum", bufs=psbufs, space=bass.MemorySpace.PSUM) as pspool,
            tc.tile_pool(name="wps", bufs=1, space=bass.MemorySpace.PSUM) as wpool,
        ):
            lt_sb = cpool.tile([TB, TB], F32, tag="lt")
            m1t_sb = cpool.tile([TB, TB], F32R, tag="m1t")
            nc.sync.dma_start(lt_sb[:], lt_dram[:])
            # SWDGE dma casts fp32 -> fp32r (the verifier requires fp32r
            # matmul operands to be produced pre-rounded)
            nc.gpsimd.dma_start(m1t_sb[:], m1t_dram[:])

            # PE warmup: dummy matmuls on the const tile while the first
            # y-group DMA is in flight, so real matmuls start at full clock
            # (HAM needs ~3us of continuous PE activity).
            if warmup:
                wps = wpool.tile([TB, TB], F32)
                for _ in range(warmup):
                    nc.tensor.matmul(
                        wps[:], lt_sb[:], lt_sb[:], start=True, stop=True
                    )

            zprev = None
            for g in range(NGL):
                rows = slice(g * GKL * TB, (g + 1) * GKL * TB)
                y_t = ypool.tile([TB, GKL, DC], F32)
                nc.sync.dma_start(
                    y_t[:], y_dram[rows, :].rearrange("(k p) d -> p k d", k=GKL, p=TB)
                )
                ogk = out_gk or GKL
                o_t = None
                for k in range(GKL):
                    if k % ogk == 0:
                        o_t = opool.tile([TB, ogk, DC], F32)
                    ko = k % ogk
                    b = g * GKL + k
                    # block-level EMA of inputs: z_b = y_b + a^128 * z_{b-1}
                    # (split into independent column chunks to shorten the
                    # serial chain; emitted first so DVE dispatches it early)
                    zcur = None
                    if 0 < b < NB - 1:
                        z_t = zpool.tile([TB, DC], F32R)
                        zw = DC // zsplit
                        for zi in range(zsplit):
                            cols = slice(zi * zw, (zi + 1) * zw)
                            nc.vector.scalar_tensor_tensor(
                                z_t[:, cols],
                                zprev[:, cols],
                                c,
                                y_t[:, k, cols],
                                op0=mybir.AluOpType.mult,
                                op1=mybir.AluOpType.add,
                            )
                        zcur = z_t[:]
                    elif b == 0:
                        z_t = zpool.tile([TB, DC], F32R)
                        nc.vector.tensor_copy(z_t[:], y_t[:, 0, :])
                        zcur = z_t[:]
                    for n0 in (0, NC_CHUNK):
                        ps = pspool.tile([TB, NC_CHUNK], F32)
                        rhs_y = y_t[:, k, n0 : n0 + NC_CHUNK]
                        if b == 0:
                            nc.tensor.matmul(ps[:], lt_sb[:], rhs_y, start=True, stop=True)
                        else:
                            # carry matmul in fp32r: its term is ~18% of the
                            # output norm, so tf32-class precision here costs
                            # ~3e-5 total rel err but runs 4x faster on PE.
                            nc.tensor.matmul(
                                ps[:],
                                m1t_sb[:],
                                zprev[:, n0 : n0 + NC_CHUNK],
                                start=True,
                                stop=False,
                            )
                            nc.tensor.matmul(ps[:], lt_sb[:], rhs_y, start=False, stop=True)
                        dst = o_t[:, ko, n0 : n0 + NC_CHUNK]
                        if dve_frac and (2 * b + (n0 != 0)) % (dve_frac + 1) < dve_frac:
                            nc.vector.tensor_copy(dst, ps[:])
                        else:
                            nc.scalar.copy(dst, ps[:])
                    if zcur is not None:
                        zprev = zcur
                    if k % ogk == ogk - 1:
                        r0 = (g * GKL + k - ogk + 1) * TB
                        orows = slice(r0, r0 + ogk * TB)
                        nc.sync.dma_start(
                            out_dram[orows, :].rearrange(
                                "(k p) d -> p k d", k=ogk, p=TB
                            ),
                            o_t[:],
                        )

    nc.compile()
    _CACHE[key] = nc
    return nc


def kernel(y_seq):
    y_seq = np.asarray(y_seq, dtype=np.float32)
    assert y_seq.shape == (B, S, D), y_seq.shape
    LT, M1T, _ = _consts()
    nc = _build()

    in_maps = []
    for core in range(NCORES):
        b, h = divmod(core, 2)
        shard = np.ascontiguousarray(y_seq[b, :, h * DC : (h + 1) * DC])
        in_maps.append({"y": shard, "lt": LT, "m1t": M1T})

    res = bass_utils.run_bass_kernel_spmd(nc, in_maps, core_ids=list(range(NCORES)))

    out = np.empty((B, S, D), dtype=np.float32)
    for core in range(NCORES):
        b, h = divmod(core, 2)
        out[b, :, h * DC : (h + 1) * DC] = res.results[core]["out"]
    return out


# revision 14
# speedup vs baseline: 1.0458x; 1.0458x over previous
"""EMA scan kernel for Trainium2 (Bass/Tile), 8-core SPMD.

Problem: h_t = (1-a)*y_t + a*h_{t-1}, h_{-1}=0, a=0.9, over y [B=4, S=4096, D=2048] f32.

Sharding: B(4) x D-half(2) -> 8 cores, each core handles a [S=4096, Dc=1024] slab.

Per-core algorithm (exact, matmul-based):
  Split S into 32 blocks of TB=128 rows. For block b:
      h_b = L @ y_b + M1 @ z_{b-1}
  where L[t,j]  = (1-a)*a^(t-j) for t>=j else 0          (in-block causal scan)
        M1[t,j] = (1-a)*a^(t+128-j)                      (previous-block window)
  and z_b = y_b + a^128 * z_{b-1} is a block-level EMA of the raw inputs.
  This is exact because the lag-(128m) window matrices satisfy M_m = a^(128(m-1)) * M1,
  so summing M_m @ y_{b-m} over all m telescopes into M1 @ z_{b-1}.

Both matmuls run on the PE (fp32, K=128, N=512 chunks, PSUM accumulate),
z-chain is one scalar_tensor_tensor per block on DVE, PSUM->SBUF copies are
split between DVE and ACT, DMA is batched 4 blocks (2 MiB) per transfer.
"""

import numpy as np

import concourse.bass as bass
import concourse.tile as tile
from concourse import bacc, mybir
from concourse import bass_utils

ALPHA = 0.9
B, S, D = 4, 4096, 2048
NCORES = 8
DC = D // 2          # per-core D chunk (1024)
TB = 128             # S-block size (partition dim)
NB = S // TB         # 32 blocks
GK = 4               # blocks per DMA group
NG = NB // GK        # 8 groups
NC_CHUNK = 512       # matmul moving-operand chunk (one PSUM bank, fp32)
F32 = mybir.dt.float32
F32R = mybir.dt.float32r  # tf32-class PE fast path (1 cyc/row vs 4 for fp32)


def _consts():
    a = ALPHA
    t = np.arange(TB)
    diff = t[:, None] - t[None, :]
    L = np.where(diff >= 0, (1.0 - a) * a ** np.maximum(diff, 0), 0.0)
    M1 = (1.0 - a) * a ** (t[:, None] + TB - t[None, :])
    LT = np.ascontiguousarray(L.T).astype(np.float32)
    M1T = np.ascontiguousarray(M1.T).astype(np.float32)
    c = float(a**TB)
    return LT, M1T, c


_CACHE = {}


def _build(ybufs=4, obufs=5, zbufs=3, psbufs=4, gk=GK, dve_frac=0, warmup=9, zsplit=4, out_gk=2, out_eng='scalar'):
    key = (ybufs, obufs, zbufs, psbufs, gk, dve_frac, warmup, zsplit, out_gk, out_eng)
    if key in _CACHE:
        return _CACHE[key]
    _, _, c = _consts()
    GKL = gk
    NGL = NB // gk

    nc = bacc.Bacc(
        "TRN2",
        target_bir_lowering=False,
        debug=False,
        enable_asserts=False,
        num_devices=NCORES,
    )
    y_dram = nc.dram_tensor("y", [S, DC], F32, kind="ExternalInput")
    lt_dram = nc.dram_tensor("lt", [TB, TB], F32, kind="ExternalInput")
    m1t_dram = nc.dram_tensor("m1t", [TB, TB], F32, kind="ExternalInput")
    out_dram = nc.dram_tensor("out", [S, DC], F32, kind="ExternalOutput")

    with tile.TileContext(nc) as tc:
        with (
            tc.tile_pool(name="consts", bufs=1) as cpool,
            tc.tile_pool(name="ypool", bufs=ybufs) as ypool,
            tc.tile_pool(name="zpool", bufs=zbufs) as zpool,
            tc.tile_pool(name="opool", bufs=obufs) as opool,
            tc.tile_pool(name="psum", bufs=psbufs, space=bass.MemorySpace.PSUM) as pspool,
            tc.tile_pool(name="wps", bufs=1, space=bass.MemorySpace.PSUM) as wpool,
        ):
            lt_sb = cpool.tile([TB, TB], F32, tag="lt")
            m1t_sb = cpool.tile([TB, TB], F32R, tag="m1t")
            nc.sync.dma_start(lt_sb[:], lt_dram[:])
            # SWDGE dma casts fp32 -> fp32r (the verifier requires fp32r
            # matmul operands to be produced pre-rounded)
            nc.gpsimd.dma_start(m1t_sb[:], m1t_dram[:])

            # PE warmup: dummy matmuls on the const tile while the first
            # y-group DMA is in flight, so real matmuls start at full clock
            # (HAM needs ~3us of continuous PE activity).
            if warmup:
                wps = wpool.tile([TB, TB], F32)
                for _ in range(warmup):
                    nc.tensor.matmul(
                        wps[:], lt_sb[:], lt_sb[:], start=True, stop=True
                    )

            zprev = None
            for g in range(NGL):
                rows = slice(g * GKL * TB, (g + 1) * GKL * TB)
                y_t = ypool.tile([TB, GKL, DC], F32)
                nc.sync.dma_start(
                    y_t[:], y_dram[rows, :].rearrange("(k p) d -> p k d", k=GKL, p=TB)
                )
                ogk = out_gk or GKL
                o_t = None
                for k in range(GKL):
                    if k % ogk == 0:
                        o_t = opool.tile([TB, ogk, DC], F32)
                    ko = k % ogk
                    b = g * GKL + k
                    # block-level EMA of inputs: z_b = y_b + a^128 * z_{b-1}
                    # (split into independent column chunks to shorten the
                    # serial chain; emitted first so DVE dispatches it early)
                    zcur = None
                    if 0 < b < NB - 1:
                        z_t = zpool.tile([TB, DC], F32R)
                        zw = DC // zsplit
                        for zi in range(zsplit):
                            cols = slice(zi * zw, (zi + 1) * zw)
                            nc.vector.scalar_tensor_tensor(
                                z_t[:, cols],
                                zprev[:, cols],
                                c,
                                y_t[:, k, cols],
                                op0=mybir.AluOpType.mult,
                                op1=mybir.AluOpType.add,
                            )
                        zcur = z_t[:]
                    elif b == 0:
                        z_t = zpool.tile([TB, DC], F32R)
                        nc.vector.tensor_copy(z_t[:], y_t[:, 0, :])
                        zcur = z_t[:]
                    for n0 in (0, NC_CHUNK):
                        ps = pspool.tile([TB, NC_CHUNK], F32)
                        rhs_y = y_t[:, k, n0 : n0 + NC_CHUNK]
                        if b == 0:
                            nc.tensor.matmul(ps[:], lt_sb[:], rhs_y, start=True, stop=True)
                        else:
                            # carry matmul in fp32r: its term is ~18% of the
                            # output norm, so tf32-class precision here costs
                            # ~3e-5 total rel err but runs 4x faster on PE.
                            nc.tensor.matmul(
                                ps[:],
                                m1t_sb[:],
                                zprev[:, n0 : n0 + NC_CHUNK],
                                start=True,
                                stop=False,
                            )
                            nc.tensor.matmul(ps[:], lt_sb[:], rhs_y, start=False, stop=True)
                        dst = o_t[:, ko, n0 : n0 + NC_CHUNK]
                        if dve_frac and (2 * b + (n0 != 0)) % (dve_frac + 1) < dve_frac:
                            nc.vector.tensor_copy(dst, ps[:])
                        else:
                            nc.scalar.copy(dst, ps[:])
                    if zcur is not None:
                        zprev = zcur
                    if k % ogk == ogk - 1:
                        r0 = (g * GKL + k - ogk + 1) * TB
                        orows = slice(r0, r0 + ogk * TB)
                        out_engine = nc.scalar if out_eng == 'scalar' else nc.sync
                        out_engine.dma_start(
                            out_dram[orows, :].rearrange(
                                "(k p) d -> p k d", k=ogk, p=TB
                            ),
                            o_t[:],
                        )

    nc.compile()
    _CACHE[key] = nc
    return nc


def kernel(y_seq):
    y_seq = np.asarray(y_seq, dtype=np.float32)
    assert y_seq.shape == (B, S, D), y_seq.shape
    LT, M1T, _ = _consts()
    nc = _build()

    in_maps = []
    for core in range(NCORES):
        b, h = divmod(core, 2)
        shard = np.ascontiguousarray(y_seq[b, :, h * DC : (h + 1) * DC])
        in_maps.append({"y": shard, "lt": LT, "m1t": M1T})

    try:
        res = bass_utils.run_bass_kernel_spmd(
            nc, in_maps, core_ids=list(range(NCORES))
        )
    except Exception:
        # transient NRT/device hiccups (e.g. first-exec unrecoverable state)
        # have been observed to succeed on retry
        res = bass_utils.run_bass_kernel_spmd(
            nc, in_maps, core_ids=list(range(NCORES))
        )

    out = np.empty((B, S, D), dtype=np.float32)
    for core in range(NCORES):
        b, h = divmod(core, 2)
        out[b, :, h * DC : (h + 1) * DC] = res.results[core]["out"]
    return out
